# revision 18
# baseline (speedup 1.0000x reference)
"""Multi-head causal attention with RoPE on 8 Trainium2 NeuronCores.

Sharding: tensor-parallel over heads x data-parallel over batch.
Core c handles batch b = c//4 and heads [4*(c%4), 4*(c%4)+4) (Hl=256 of Hd=1024).
Each core computes q/k/v projections for its head slice (column-split Wq/Wk/Wv),
RoPE, causal softmax attention, and a partial output projection (row-split Wo).
The host sums the 4 partial outputs per batch (the "all-reduce").

Device layouts (per core, S=2048, E=1024, Hl=256, D=64):
  xT   [E, S]    x transposed (host-side) so E rides the partition dim
  qT/kT slabs [128, S] x2: partitions = 2 heads x 64 dims, free = seq
  v    16 tiles [128, 260]: partitions = seq chunk, free = 4 heads x (64 dims + ones col)
  scores computed transposed (keys on partitions), softmax Z via ones-column of v,
  normalization by 1/Z broadcast via a DRAM-roundtrip partition-broadcast DMA.

The attention phase is ACT(exp)-bound, so the PE fillers -- late v-projection
chunks and the per-block output projection -- are interleaved into the
attention stream as pipeline items that borrow score-ring PSUM slots. Output
partials stream to DRAM as fp16 per 128-row chunk while attention continues.
"""
import sys

sys.path.insert(0, "/opt/trn_rl_repo")
import numpy as np  # noqa: E402

N_HEADS = 16
B, S, E, HD = 2, 2048, 1024, 1024
D = HD // N_HEADS  # 64
HPC = 4            # heads per core
HL = HPC * D       # 256
NCORES = 8
ROPE_BASE = 10000.0

_built = None


def _build_nc():
    import concourse.bass as bass
    import concourse.tile as tile
    from concourse import bacc, mybir

    F32 = mybir.dt.float32
    F32R = mybir.dt.float32r
    F16 = mybir.dt.float16
    Exp = mybir.ActivationFunctionType.Exp
    is_ge = mybir.AluOpType.is_ge
    ts = bass.ts

    nc = bacc.Bacc("TRN2", target_bir_lowering=False, debug=False)
    xT_d = nc.dram_tensor("xT", [E, S], F16, kind="ExternalInput").ap()
    wq_d = nc.dram_tensor("wq", [E, HL], F16, kind="ExternalInput").ap()
    wk_d = nc.dram_tensor("wk", [E, HL], F16, kind="ExternalInput").ap()
    wv_d = nc.dram_tensor("wv", [E, HL], F16, kind="ExternalInput").ap()
    wo_d = nc.dram_tensor("wo", [HL, E], F16, kind="ExternalInput").ap()
    cos_d = nc.dram_tensor("cosx", [128, S], F16, kind="ExternalInput").ap()
    sin_d = nc.dram_tensor("sinx", [128, S], F16, kind="ExternalInput").ap()
    out_d = nc.dram_tensor("out", [S, E], F16, kind="ExternalOutput").ap()

    ECH = E // 128   # 8 e-chunks
    SCH = S // 128   # 16 seq chunks
    SB = S // 512    # 4 seq blocks
    swap_mask = []
    for i in range(16):
        swap_mask += [2 * i + 1, 2 * i]

    with tile.TileContext(nc) as tc:
        with (
            tc.tile_pool(name="persist", bufs=1) as pp,
            tc.tile_pool(name="evict", bufs=6) as ev,
            tc.tile_pool(name="bxv", bufs=1) as bxv,
        ):
            # persistent tiles
            qT = [pp.tile([128, S], F16, tag=f"qT{c}", name=f"qT{c}") for c in range(2)]
            kT = [pp.tile([128, S], F16, tag=f"kT{c}", name=f"kT{c}") for c in range(2)]
            vt = [pp.tile([128, HPC * (D + 1)], F16, tag=f"v{t}", name=f"v{t}")
                  for t in range(SCH)]
            oT = [pp.tile([128, S], F16, tag=f"oT{c}", name=f"oT{c}") for c in range(2)]
            cosx = pp.tile([128, S], F16, tag="cosx", name="cosx")
            sinx = pp.tile([128, S], F16, tag="sinx", name="sinx")
            wo_t = pp.tile([128, 2, E], F16, tag="wo", name="wo")
            onesr = pp.tile([65, 128], F16, tag="ones", name="onesr")
            # x and Wv live through the attention phase: late v-projection
            # chunks are computed there as PE fillers
            wv_t = bxv.tile([128, ECH, HL], F16, tag="wv", name="wv")
            xt = [bxv.tile([128, S], F16, tag=f"x{e}", name=f"x{e}")
                  for e in range(ECH)]

            def vchunk_mm(t, psreg):
                nc.gpsimd.memset(
                    vt[t].rearrange("p (h c) -> p h c", c=D + 1)[:, :, D:D + 1],
                    1.0,
                )
                for e in range(ECH):
                    nc.tensor.matmul(
                        psreg,
                        xt[e][:, ts(t, 128)],
                        wv_t[:, e, :],
                        start=(e == 0),
                        stop=(e == ECH - 1),
                    )

            def vchunk_evict(t, psreg):
                nc.vector.tensor_copy(
                    out=vt[t].rearrange("p (h c) -> p h c", c=D + 1)[:, :, 0:D],
                    in_=psreg.rearrange("p (h c) -> p h c", c=D),
                )

            # ---------------- Phase B: q/k projections + RoPE, v chunks 0-3 ---
            with (
                tc.tile_pool(name="bq", bufs=1) as bq,
                tc.tile_pool(name="bswp", bufs=2) as bswp,
                tc.tile_pool(name="bps", bufs=8, space="PSUM") as bps,
            ):
                wq_t = bq.tile([128, ECH, HL], F16, tag="wq", name="wq")
                wk_t = bq.tile([128, ECH, HL], F16, tag="wk", name="wk")

                def xdma(eng, e, j):
                    eng.dma_start(
                        out=xt[e][:, ts(j, 512)],
                        in_=xT_d[e * 128:(e + 1) * 128, ts(j, 512)],
                    )
                # DMA dealt round-robin over the sync/scalar/gpsimd queues in
                # strict need-order (per-queue dispatch serializes at ~0.8us
                # and per-queue transfers serialize, so order = arrival time)
                nc.gpsimd.memset(onesr[:], 1.0)
                qrr = [0]

                def deal():
                    eng = (nc.sync, nc.scalar, nc.gpsimd)[qrr[0] % 3]
                    qrr[0] += 1
                    return eng

                def whalf(w_t_, w_d_, h):
                    deal().dma_start(
                        out=w_t_[:, 4 * h:4 * h + 4, :],
                        in_=w_d_.rearrange("(c p) m -> p c m", p=128)
                        [:, 4 * h:4 * h + 4, :],
                    )
                whalf(wq_t, wq_d, 0)
                whalf(wq_t, wq_d, 1)
                for e in range(ECH):
                    xdma(deal(), e, 0)
                whalf(wk_t, wk_d, 0)
                whalf(wk_t, wk_d, 1)
                for e in range(ECH):
                    xdma(deal(), e, 1)
                deal().dma_start(out=cosx[:], in_=cos_d)
                deal().dma_start(out=sinx[:], in_=sin_d)
                for j in range(2, SB):
                    for e in range(ECH):
                        xdma(deal(), e, j)
                whalf(wv_t, wv_d, 0)
                whalf(wv_t, wv_d, 1)
                deal().dma_start(
                    out=wo_t[:],
                    in_=wo_d.rearrange("(c p) e -> p c e", p=128),
                )

                # q/k projections -> transposed slabs; evictions ride the ACT
                # engine (idle in this phase) and the RoPE runs per 512-wide
                # block on the DVE right after each eviction, so slab-1 ropes
                # finish before the attention stream needs them
                def rope_blk(dest, c, j):
                    sw = bswp.tile([128, 512], F16, tag="swp", name="swp")
                    dj = dest[c][:, ts(j, 512)]
                    nc.vector.stream_shuffle(out=sw[:], in_=dj, mask=swap_mask)
                    nc.vector.tensor_mul(
                        out=sw[:], in0=sw[:], in1=sinx[:, ts(j, 512)]
                    )
                    nc.vector.tensor_mul(
                        out=dj, in0=dj, in1=cosx[:, ts(j, 512)]
                    )
                    nc.vector.tensor_add(out=dj, in0=dj, in1=sw[:])

                def qkgroup(w_t_, dest, m, j):
                    ps = bps.tile([128, 512], F32, tag="mm", name="mm")
                    for e in range(ECH):
                        nc.tensor.matmul(
                            ps[:],
                            w_t_[:, e, m * 128:(m + 1) * 128],
                            xt[e][:, ts(j, 512)],
                            start=(e == 0),
                            stop=(e == ECH - 1),
                        )
                    nc.scalar.copy(out=dest[m][:, ts(j, 512)], in_=ps[:])
                    rope_blk(dest, m, j)

                for m in range(2):
                    for j in range(SB):
                        qkgroup(wq_t, qT, m, j)
                        qkgroup(wk_t, kT, m, j)
                # v chunks 0-3 (block 0); the rest interleave into attention
                for t in range(4):
                    ps = bps.tile([128, HL], F32, tag="mm", name="mmv")
                    vchunk_mm(t, ps[:])
                    vchunk_evict(t, ps[:])

            # -------- Phase C: attention + interleaved v-proj and out-proj ----
            with (
                tc.tile_pool(name="cexp", bufs=6) as cexp,
                tc.tile_pool(name="cz", bufs=4) as cz,
                tc.tile_pool(name="crb", bufs=3) as crb,
                tc.tile_pool(name="csc", bufs=3, space="PSUM") as csc,
                tc.tile_pool(name="cpv", bufs=1, space="PSUM") as cpv,
            ):
                def qksv(c):
                    hs = [2 * c, 2 * c + 1]
                    qs = [qT[c][0:64, :], qT[c][64:128, :]]
                    ks = [kT[c][0:64, :], kT[c][64:128, :]]
                    vs = [
                        [vt[t].rearrange("p (h c) -> p h c", c=D + 1)[:, h, :]
                         for t in range(SCH)]
                        for h in hs
                    ]
                    return hs, qs, ks, vs

                # exact per-chunk causal trim (stale-PSUM columns that the
                # trims skip are exp'd but never consumed by the pv matmuls)
                def off_of(t, j):
                    return max(t * 128 - j * 512, 0)

                # the pipeline stream: attention chunk-pair units ("A"),
                # v-projection fillers ("V"), out-projection chunks ("D");
                # each item emits matmuls one slot ahead of its consumption
                stream = []
                for c in range(2):
                    stream.append(("A", c, 0, 0))
                    stream.append(("A", c, 0, 1))
                for j in range(1, SB):
                    blk = []
                    for c in range(2):
                        for tp in range(2 * (j + 1)):
                            blk.append(("A", c, j, tp))
                    blk.insert(1, ("V", j, 0))
                    blk.insert(2, ("V", j, 1))
                    for idx in range(4):
                        blk.insert(5 + 2 * idx, ("D", j - 1, idx))
                    stream += blk
                # the last block's out-projection runs after the main loop:
                # its oT writes are only emitted in the final A-consume, so
                # emitting these matmuls earlier would race the normalization
                tail = [("D", SB - 1, idx) for idx in range(4)]

                sc_of = {}
                pv_of = {}
                dma_rr = [0]

                def emit_mm(u):
                    kind = u[0]
                    if kind == "A":
                        _, c, j, tp = u
                        _, qs, ks, _ = qksv(c)
                        sc = [csc.tile([128, 1024], F32, tag="sc", name="sc")
                              for _ in range(2)]
                        # head 0 on PE rows 0-63, head 1 on rows 64-127
                        for half in range(2):
                            t = 2 * tp + half
                            off = off_of(t, j)
                            for i in range(2):
                                nc.tensor.matmul(
                                    sc[i][:, half * 512 + off:(half + 1) * 512],
                                    ks[i][:, ts(t, 128)],
                                    qs[i][:, j * 512 + off:(j + 1) * 512],
                                    start=True,
                                    stop=True,
                                )
                        sc_of[u] = sc
                    elif kind == "V":
                        _, j, h = u
                        slot = csc.tile([128, 1024], F32, tag="sc", name="vps")
                        for q in range(2):
                            vchunk_mm(4 * j + 2 * h + q,
                                      slot[:, 512 * q:512 * q + HL])
                        sc_of[u] = slot
                    else:  # "D": out projection for seq chunk 4j+tt
                        _, j, tt = u
                        t = 4 * j + tt
                        slot = csc.tile([128, 1024], F32, tag="sc", name="dps")
                        for n in range(2):
                            for c in range(2):
                                nc.tensor.matmul(
                                    slot[:, n * 512:(n + 1) * 512],
                                    oT[c][:, ts(t, 128)],
                                    wo_t[:, c, ts(n, 512)],
                                    start=(c == 0),
                                    stop=(c == 1),
                                )
                        sc_of[u] = slot

                def emit_consume(u):
                    kind = u[0]
                    if kind == "V":
                        _, j, h = u
                        slot = sc_of.pop(u)
                        for q in range(2):
                            vchunk_evict(4 * j + 2 * h + q,
                                         slot[:, 512 * q:512 * q + HL])
                        return
                    if kind == "D":
                        _, j, tt = u
                        t = 4 * j + tt
                        slot = sc_of.pop(u)
                        of = ev.tile([128, 1024], F16, tag="out", name="oev")
                        nc.vector.tensor_copy(out=of[:], in_=slot[:])
                        eng = (nc.sync, nc.gpsimd)[dma_rr[0] % 2]
                        dma_rr[0] += 1
                        eng.dma_start(out=out_d[ts(t, 128), :], in_=of[:])
                        return
                    _, c, j, tp = u
                    nt = 4 * (j + 1)
                    hs, _, _, vs = qksv(c)
                    if tp == 0:
                        pv_of[(c, j)] = [
                            cpv.tile([65, 512], F32, tag=f"pv{i}", name=f"pv{i}")
                            for i in range(2)
                        ]
                    pv = pv_of[(c, j)]
                    sc = sc_of.pop(u)
                    # last pair of a block is mostly masked: exp only the
                    # computed slices; earlier pairs exp full width
                    lastpair = (tp == nt // 2 - 1)
                    exm = []
                    for i in range(2):
                        ex = cexp.tile([128, 1024], F16, tag="ex", name="ex")
                        if lastpair:
                            for half in range(2):
                                off = off_of(2 * tp + half, j)
                                nc.scalar.activation(
                                    out=ex[:, half * 512 + off:(half + 1) * 512],
                                    in_=sc[i][:, half * 512 + off:(half + 1) * 512],
                                    func=Exp, scale=0.125,
                                )
                        else:
                            nc.scalar.activation(
                                out=ex[:], in_=sc[i][:], func=Exp, scale=0.125
                            )
                        exm.append(ex)
                    for half in range(2):
                        t = 2 * tp + half
                        off = off_of(t, j)
                        if t >= nt - 4:  # diagonal chunk: causal mask
                            for i in range(2):
                                nc.gpsimd.affine_select(
                                    out=exm[i][:, half * 512 + off:
                                               (half + 1) * 512],
                                    in_=exm[i][:, half * 512 + off:
                                              (half + 1) * 512],
                                    compare_op=is_ge,
                                    fill=0.0,
                                    base=(j * 512 - t * 128) + off,
                                    channel_multiplier=-1,
                                    pattern=[[1, 512 - off]],
                                )
                        for i in range(2):
                            nc.tensor.matmul(
                                pv[i][:, off:512],
                                vs[i][t],
                                exm[i][:, half * 512 + off:(half + 1) * 512],
                                start=(t == 0),
                                stop=(t == nt - 1),
                            )
                    if tp == nt // 2 - 1:
                        # end of block: evict + per-block softmax normalization.
                        # Z rides to all partitions via a ones-vector matmul
                        # into a borrowed score-ring slot -- much shorter
                        # latency than a DMA broadcast roundtrip. The finish
                        # (bcast+recip+mul) is deferred two stream items so
                        # the PE never queues behind the DVE Z copies.
                        zq = cz.tile([65, 2, 512], F16, tag="zq", name="zq")
                        for i in range(2):
                            nc.vector.tensor_copy(
                                out=oT[c][i * 64:(i + 1) * 64, ts(j, 512)],
                                in_=pv[i][0:64, :],
                            )
                            nc.vector.tensor_copy(
                                out=zq[64:65, i, :], in_=pv[i][64:65, :]
                            )

                        def norm_finish(c=c, j=j, zq=zq):
                            rbps = csc.tile([128, 1024], F32, tag="sc",
                                            name="rbps")
                            for i in range(2):
                                nc.tensor.matmul(
                                    rbps[i * 64:(i + 1) * 64, 0:512],
                                    onesr[64:65, 0:64],
                                    zq[64:65, i, :],
                                    start=True,
                                    stop=True,
                                )
                            rbr = crb.tile([128, 512], F32, tag="rbr",
                                           name="rbr")
                            nc.vector.reciprocal_approx_fast(
                                out=rbr[:], in_=rbps[:, 0:512]
                            )
                            nc.vector.tensor_mul(
                                out=oT[c][:, ts(j, 512)],
                                in0=oT[c][:, ts(j, 512)],
                                in1=rbr[:],
                            )
                        deferred.append([2, norm_finish])

                deferred = []

                def run_deferred(flush=False):
                    rest = []
                    for item in deferred:
                        item[0] -= 1
                        if flush or item[0] <= 0:
                            item[1]()
                        else:
                            rest.append(item)
                    deferred[:] = rest

                emit_mm(stream[0])
                for un in range(1, len(stream)):
                    emit_mm(stream[un])
                    emit_consume(stream[un - 1])
                    run_deferred()
                emit_consume(stream[-1])
                run_deferred(flush=True)
                emit_mm(tail[0])
                for un in range(1, len(tail)):
                    emit_mm(tail[un])
                    emit_consume(tail[un - 1])
                emit_consume(tail[-1])

    nc.compile()
    return nc


def _rope_tables():
    iexp = np.arange(0, D, 2, dtype=np.float32) / np.float32(D)
    inv_freq = np.reciprocal(np.power(np.float32(ROPE_BASE), iexp))  # (32,) f32
    ang = np.arange(S, dtype=np.float32)[:, None] * inv_freq[None, :]  # (S, 32)
    cos = np.cos(ang).astype(np.float32)  # (S, 32)
    sin = np.sin(ang).astype(np.float32)
    cosx = np.empty((64, S), dtype=np.float32)
    sinx = np.empty((64, S), dtype=np.float32)
    cosx[0::2] = cos.T
    cosx[1::2] = cos.T
    sinx[0::2] = -sin.T
    sinx[1::2] = sin.T
    return (np.tile(cosx, (2, 1)).astype(np.float16),
            np.tile(sinx, (2, 1)).astype(np.float16))  # (128, S) each


def get_nc():
    global _built
    if _built is None:
        _built = _build_nc()
    return _built


def make_in_maps(x, Wq, Wk, Wv, Wo):
    cosx, sinx = _rope_tables()
    in_maps = []
    for c in range(NCORES):
        b, g = c // 4, c % 4
        sl = slice(g * HL, (g + 1) * HL)
        in_maps.append({
            "xT": np.ascontiguousarray(x[b].T).astype(np.float16),
            "wq": np.ascontiguousarray(Wq[:, sl]).astype(np.float16),
            "wk": np.ascontiguousarray(Wk[:, sl]).astype(np.float16),
            "wv": np.ascontiguousarray(Wv[:, sl]).astype(np.float16),
            "wo": np.ascontiguousarray(Wo[sl, :]).astype(np.float16),
            "cosx": cosx,
            "sinx": sinx,
        })
    return in_maps


def gather(results):
    out = np.empty((B, S, E), dtype=np.float32)
    for b in range(B):
        acc = results[4 * b]["out"].astype(np.float32)
        for g in range(1, 4):
            acc += results[4 * b + g]["out"].astype(np.float32)
        out[b] = acc
    return out


def kernel(x, Wq, Wk, Wv, Wo):
    from concourse.bass_utils import run_bass_kernel_spmd

    nc = get_nc()
    in_maps = make_in_maps(
        np.asarray(x), np.asarray(Wq), np.asarray(Wk), np.asarray(Wv), np.asarray(Wo)
    )
    res = run_bass_kernel_spmd(nc, in_maps, list(range(NCORES)))
    return gather(res.results)
